# revision 17
# baseline (speedup 1.0000x reference)
"""Trainium2 Bass kernel for nn_Encoder_37915971289796 (6-layer transformer encoder).

Strategy: pure data-parallel over batch (B=16 -> 2 per core, 8 cores, no
collectives). Per core, activations live feature-major in SBUF
([D=1024 across 8x128-partition tiles, 1024 tokens]); weights are host-cast to
bf16 and streamed once per layer; matmuls run bf16 (1 cyc/row) with fp32 PSUM
accumulation. Attention uses a transposed-softmax scheme: scores are computed
as [k,q] tiles; the relative-position bias is folded in MULTIPLICATIVELY after
the exp (ex * exp(bias), exp table precomputed host-side, loaded as a Toeplitz
window with all-positive DMA strides and consumed through a reversed free-axis
AP) on the otherwise-idle Vector/Pool engines; softmax denominators come from
ones-vector matmuls (partition-axis reduction on PE); no max-subtraction
(logits are provably tiny for this model scale). LayerNorm statistics are
computed with ones-matmul partition reductions, chunked by token-half so each
half's QKV can start while the other half normalizes.

Self-contained: hardcodes all shapes; takes FULL inputs, returns FULL output.
"""

import numpy as np
import ml_dtypes
from contextlib import ExitStack

import concourse.bass as bass
import concourse.mybir as mybir
import concourse.tile as tile
from concourse import bacc
from concourse.bass_utils import run_bass_kernel_spmd

F32 = mybir.dt.float32
BF16 = mybir.dt.bfloat16
AF = mybir.ActivationFunctionType
BF = ml_dtypes.bfloat16

L, D, H, F, S, B, P = 6, 1024, 16, 4096, 512, 16, 512
DH = D // H              # 64
NCORES = 8
BL = B // NCORES         # 2 batches per core
NT = BL * S              # 1024 tokens per core
DT = D // 128            # 8 d-tiles
FT = F // 128            # 32 f-tiles
TBL = 2 * P - 1          # 1023
ETW = 514                # padded per-(hh,kt) bias window width (513 used)
EPS = 1e-6
QSCALE = 1.0 / float(np.sqrt(DH))

_CACHE = {}


def _build(flags):
    """Build the per-core Bass program. flags: (use_pbias, use_obias, use_ln1, use_ln2)"""
    use_pbias, use_obias, use_ln1, use_ln2 = flags
    nc = bacc.Bacc("TRN2", target_bir_lowering=False, debug=False)

    x_d = nc.dram_tensor("x", [BL, S, D], F32, kind="ExternalInput").ap()
    pe_d = nc.dram_tensor("pe", [S, D], F32, kind="ExternalInput").ap()
    # host-relayouted weights (bf16):
    # weights pre-tiled host-side so every DMA reads one contiguous chunk per
    # partition: [..., p, kt, m] with element = W[kt*128+p, <out-slice> m]
    wqkv_d = {}
    for w in ("wq", "wk", "wv", "wo"):
        wqkv_d[w] = nc.dram_tensor(w, [L, 2, 128, DT, 512], BF16,
                                   kind="ExternalInput").ap()
    w1_d = nc.dram_tensor("w1", [L, FT, 128, DT, 128], BF16,
                          kind="ExternalInput").ap()
    w2_d = nc.dram_tensor("w2", [L, DT, 128, FT, 128], BF16,
                          kind="ExternalInput").ap()
    # exp(bias) table, transposed + reversed + 1-lead-padded: [L,H,1+TBL]
    etr_d = nc.dram_tensor("etr", [L, H, 1 + TBL], BF16, kind="ExternalInput").ap()
    imat_d = nc.dram_tensor("imat", [128, 128], F32, kind="ExternalInput").ap()
    if use_pbias:  # bq*QSCALE, bk, bv, b1 applied via ACT bias APs
        pb_d = {w: nc.dram_tensor(f"b_{w}", [L, D if w != "b1" else F], F32,
                                  kind="ExternalInput").ap()
                for w in ("bq", "bk", "bv", "b1")}
    if use_obias:  # bo, b2 applied via extra DVE passes
        ob_d = {w: nc.dram_tensor(f"b_{w}", [L, D], F32, kind="ExternalInput").ap()
                for w in ("bo", "b2")}
    if use_ln1:
        ln1g_d = nc.dram_tensor("ln1_g", [L, D], F32, kind="ExternalInput").ap()
        ln1b_d = nc.dram_tensor("ln1_b", [L, D], F32, kind="ExternalInput").ap()
    if use_ln2:
        ln2g_d = nc.dram_tensor("ln2_g", [L, D], F32, kind="ExternalInput").ap()
        ln2b_d = nc.dram_tensor("ln2_b", [L, D], F32, kind="ExternalInput").ap()
    out_d = nc.dram_tensor("out", [BL, S, D], F32, kind="ExternalOutput").ap()

    with tile.TileContext(nc) as tc, ExitStack() as CTX, \
            nc.allow_low_precision(reason="bf16 matmul pipeline"):
        cst = CTX.enter_context(tc.tile_pool(name="cst", bufs=1))
        im = cst.tile([128, 128], F32, tag="im")
        nc.sync.dma_start(out=im, in_=imat_d)
        ones_bf = cst.tile([128, 1], BF16, tag="onesb")
        nc.vector.memset(ones_bf, 1.0)
        ones1f = cst.tile([1, 128], F32, tag="ones1")
        nc.vector.memset(ones1f, 1.0)
        ones1b = cst.tile([1, 64], BF16, tag="ones1b")
        nc.vector.memset(ones1b, 1.0)
        epsb = cst.tile([1, 1], F32, tag="epsb")
        nc.vector.memset(epsb, EPS)

        hp = CTX.enter_context(tc.tile_pool(name="hp", bufs=1))
        h = hp.tile([128, DT, NT], F32, tag="h")

        ap_pool = CTX.enter_context(tc.tile_pool(name="apool", bufs=1))   # slot A
        bp_pool = CTX.enter_context(tc.tile_pool(name="bpool", bufs=1))   # slot B
        wp = CTX.enter_context(tc.tile_pool(name="wp", bufs=4))           # weights
        psA = CTX.enter_context(tc.tile_pool(name="psA", bufs=3, space="PSUM"))
        psB = CTX.enter_context(tc.tile_pool(name="psB", bufs=3, space="PSUM"))
        psS = CTX.enter_context(tc.tile_pool(name="psS", bufs=2, space="PSUM"))

        # ---------------- input prep: h = (x + pe)^T feature-major ----------
        with ExitStack() as SP:
            pp = SP.enter_context(tc.tile_pool(name="prep", bufs=5))
            for b in range(BL):
                xt = []
                for st in range(4):
                    xpe = pp.tile([128, D], F32, tag="xpe", name=f"xpe{b}{st}")
                    nc.sync.dma_start(out=xpe, in_=x_d[b, st * 128:(st + 1) * 128, :])
                    pet = pp.tile([128, D], F32, tag="pet", name=f"pet{b}{st}")
                    nc.sync.dma_start(out=pet, in_=pe_d[st * 128:(st + 1) * 128, :])
                    nc.vector.tensor_add(xpe, xpe, pet)
                    xt.append(xpe)
                for d in range(DT):
                    ps = psA.tile([128, 512], F32, tag="pa", name=f"prtp{b}{d}")
                    for j in range(4):
                        nc.tensor.transpose(
                            ps[:, j * 128:(j + 1) * 128],
                            xt[j][:, d * 128:(d + 1) * 128], im)
                    nc.scalar.copy(h[:, d, b * 512:(b + 1) * 512], ps)

        # ---------------- helpers -------------------------------------------
        def layernorm(src, dst, li, g_d, b_d, use_aff, tagp):
            """src [128,DT,NT] f32 -> dst [128,DT,NT] bf16 (normalized).

            Processed per token-half (ch) so downstream consumers of half 0
            can start while half 1 is still normalizing.
            """
            with ExitStack() as SL:
                sp = SL.enter_context(tc.tile_pool(name=f"ln{tagp}", bufs=1))
                rp = SL.enter_context(tc.tile_pool(name=f"lnr{tagp}", bufs=2))
                if use_aff:
                    gsb = rp.tile([128, DT], F32, tag="gsb", bufs=1)
                    nc.sync.dma_start(out=gsb, in_=bass.AP(
                        tensor=g_d.tensor, offset=li * D, ap=[[1, 128], [128, DT]]))
                    bsb = rp.tile([128, DT], F32, tag="bsb", bufs=1)
                    nc.sync.dma_start(out=bsb, in_=bass.AP(
                        tensor=b_d.tensor, offset=li * D, ap=[[1, 128], [128, DT]]))
                t0p = SL.enter_context(tc.tile_pool(name=f"lnt{tagp}", bufs=2))
                for ch in range(2):
                    csl = slice(ch * 512, (ch + 1) * 512)
                    hbf = sp.tile([128, DT, 512], BF16, tag="hbf", name=f"hbf{tagp}{ch}")
                    nc.scalar.copy(hbf, src[:, :, csl])
                    hsq = sp.tile([128, DT, 512], BF16, tag="hsq", name=f"hsq{tagp}{ch}")
                    nc.vector.tensor_mul(hsq, hbf, hbf)
                    mu = rp.tile([1, 512], F32, tag="mu", name=f"mu{tagp}{ch}")
                    e2 = rp.tile([1, 512], F32, tag="e2", name=f"e2{tagp}{ch}")
                    for (srct, dstr) in ((hbf, mu), (hsq, e2)):
                        ssp = psS.tile([1, 512], F32, tag="ps",
                                       name=f"st{tagp}{ch}{dstr.name}")
                        for k in range(DT):
                            nc.tensor.matmul(ssp, ones_bf, srct[:, k, :],
                                             start=(k == 0), stop=(k == DT - 1))
                        nc.scalar.activation(dstr, ssp, AF.Copy, scale=1.0 / D)
                    msq = rp.tile([1, 512], F32, tag="msq", name=f"msq{tagp}{ch}")
                    nc.scalar.square(msq, mu)
                    var = rp.tile([1, 512], F32, tag="var", name=f"var{tagp}{ch}")
                    nc.vector.tensor_sub(var, e2, msq)
                    lnv = rp.tile([1, 512], F32, tag="lnv", name=f"lnv{tagp}{ch}")
                    nc.scalar.activation(lnv, var, AF.Ln, bias=epsb)
                    rstd = rp.tile([1, 512], F32, tag="rstd", name=f"rstd{tagp}{ch}")
                    nc.scalar.activation(rstd, lnv, AF.Exp, scale=-0.5)
                    ms = rp.tile([1, 512], F32, tag="ms", name=f"ms{tagp}{ch}")
                    nc.vector.tensor_mul(ms, mu, rstd)
                    rstd_b = rp.tile([128, 512], F32, tag="rstdb", name=f"rb{tagp}{ch}")
                    ms_b = rp.tile([128, 512], F32, tag="msb", name=f"mb{tagp}{ch}")
                    for (row, bc) in ((rstd, rstd_b), (ms, ms_b)):
                        bps = psA.tile([128, 512], F32, tag="pa",
                                       name=f"bc{tagp}{ch}{bc.name}")
                        nc.tensor.matmul(bps, ones1f, row, start=True, stop=True)
                        nc.scalar.copy(bc, bps)
                    for k in range(DT):
                        # split the normalize across Vector and Pool so the
                        # first half's QKV matmuls unblock sooner
                        eng = nc.vector if (k % 2 == 0) else nc.gpsimd
                        t0 = t0p.tile([128, 512], F32, tag="t0", name=f"t0{k}{ch}")
                        eng.tensor_mul(t0, src[:, k, csl], rstd_b)
                        if use_aff:
                            t1 = t0p.tile([128, 512], F32, tag="t1", name=f"t1{k}{ch}")
                            eng.tensor_sub(t1, t0, ms_b)
                            nc.scalar.activation(dst[:, k, csl], t1,
                                                 AF.Identity, bias=bsb[:, k:k + 1],
                                                 scale=gsb[:, k:k + 1])
                        else:
                            eng.tensor_sub(dst[:, k, csl], t0, ms_b)

        def load_bias_row(d_ap, li, width, name):
            """bias row [width] -> [128, width//128] sbuf f32 (feature-major)."""
            t = wp.tile([128, width // 128], F32, tag="w", name=name)
            nc.sync.dma_start(out=t, in_=bass.AP(
                tensor=d_ap.tensor, offset=li * width,
                ap=[[1, 128], [128, width // 128]]))
            return t

        # ---------------- layers --------------------------------------------
        for i in range(L):
            # ---- LN1 -> xn (slot B)
            xn = bp_pool.tile([128, DT, NT], BF16, tag="B", name=f"xn{i}")
            layernorm(h, xn, i, ln1g_d if use_ln1 else None,
                      ln1b_d if use_ln1 else None, use_ln1, f"a{i}")

            # ---- QKV projections -> qkv (slot A): q[0:8], k[8:16], v[16:24]
            qkv = ap_pool.tile([128, 24, NT], BF16, tag="A", name=f"qkv{i}")
            bq_sb = bk_sb = bv_sb = None
            if use_pbias:
                bq_sb = load_bias_row(pb_d["bq"], i, D, f"bq{i}")
                bk_sb = load_bias_row(pb_d["bk"], i, D, f"bk{i}")
                bv_sb = load_bias_row(pb_d["bv"], i, D, f"bv{i}")
            qk_wt = {}
            for wi, wname in enumerate(("wq", "wk")):
                for hf in range(2):
                    wt = wp.tile([128, DT, 512], BF16, tag="w", name=f"{wname}{i}{hf}")
                    nc.sync.dma_start(out=wt, in_=wqkv_d[wname][i, hf])
                    qk_wt[(wi, hf)] = wt
            for ch in range(2):   # token half: QK of half 0 unblock attention b=0
                for wi, wname in enumerate(("wq", "wk")):
                    bsb = (bq_sb, bk_sb)[wi]
                    scl = QSCALE if wname == "wq" else 1.0
                    for hf in range(2):
                        wt = qk_wt[(wi, hf)]
                        for m in range(4):
                            mg = hf * 4 + m
                            pps = psA.tile([128, 512], F32, tag="pa",
                                           name=f"p{wname}{mg}{ch}")
                            for k in range(DT):
                                nc.tensor.matmul(
                                    pps, wt[:, k, m * 128:(m + 1) * 128],
                                    xn[:, k, ch * 512:(ch + 1) * 512],
                                    start=(k == 0), stop=(k == DT - 1))
                            dsl = qkv[:, wi * 8 + mg, ch * 512:(ch + 1) * 512]
                            if use_pbias:
                                nc.scalar.activation(dsl, pps, AF.Identity,
                                                     bias=bsb[:, mg:mg + 1], scale=scl)
                            else:
                                nc.scalar.activation(dsl, pps, AF.Copy, scale=scl)
            # V: token-major out
            for hf in range(2):  # half of out-features
                wt = wp.tile([128, DT, 512], BF16, tag="w", name=f"wv{i}{hf}")
                nc.sync.dma_start(out=wt, in_=wqkv_d["wv"][i, hf])
                for mt in range(DT):  # token tile
                    pps = psA.tile([128, 512], F32, tag="pa", name=f"pv{mt}{hf}")
                    for k in range(DT):
                        nc.tensor.matmul(
                            pps, xn[:, k, mt * 128:(mt + 1) * 128],
                            wt[:, k, :], start=(k == 0), stop=(k == DT - 1))
                    dsl = qkv[:, 16 + mt, hf * 512:(hf + 1) * 512]
                    if use_pbias:
                        bvb = wp.tile([128, 512], F32, tag="w", name=f"bvb{i}{hf}")
                        nc.sync.dma_start(out=bvb, in_=bass.AP(
                            tensor=pb_d["bv"].tensor, offset=i * D + hf * 512,
                            ap=[[0, 128], [1, 512]]))
                        nc.vector.tensor_add(dsl, pps, bvb)
                    else:
                        nc.scalar.copy(dsl, pps)

            # ---- attention -> ctxT (slot B); heads processed in pairs so all
            # DVE/ACT ops span the full 128 partitions with aligned bases
            ctxT = bp_pool.tile([128, DT, NT], BF16, tag="B", name=f"ctxT{i}")
            with ExitStack() as SA:
                ap_ = SA.enter_context(tc.tile_pool(name=f"attn{i}", bufs=2))
                ep_ = SA.enter_context(tc.tile_pool(name=f"exp{i}", bufs=6))
                for th in range(H // 2):  # head pair (2*th, 2*th+1)
                    # exp(bias) Toeplitz windows; eb[p, hh, kt, j] =
                    # etr[i, 2th+hh, kt*128 + p + j], j in [0,513)
                    eb = ap_.tile([128, 2, 4, ETW], BF16, tag="eb", name=f"eb{i}{th}")
                    for hh in range(2):
                        for kt in range(4):
                            nc.sync.dma_start(out=eb[:, hh, kt, 0:513], in_=bass.AP(
                                tensor=etr_d.tensor,
                                offset=(i * H + 2 * th + hh) * (1 + TBL) + kt * 128,
                                ap=[[1, 128], [1, 513]]))
                    for b in range(BL):
                        cps = psB.tile([128, 512], F32, tag="pb", name=f"c{th}{b}")
                        rbp = psB.tile([128, 512], F32, tag="pb", name=f"rb{th}{b}")
                        exbs = {}
                        # scores: the two heads of the pair occupy disjoint
                        # row-halves of the PE array -> concurrent via
                        # tile_position row groups
                        for kt in range(4):
                            scps = {}
                            for hh in range(2):
                                ro = hh * 64
                                qs = qkv[ro:ro + 64, th, b * 512:(b + 1) * 512]
                                scp = psA.tile([128, 512], F32, tag="pa",
                                               name=f"sc{th}{b}{hh}{kt}")
                                nc.tensor.matmul(
                                    scp,
                                    qkv[ro:ro + 64, 8 + th,
                                        b * 512 + kt * 128: b * 512 + (kt + 1) * 128],
                                    qs, start=True, stop=True,
                                    tile_position=(ro, 0))
                                scps[hh] = scp
                            for hh in range(2):
                                ex = ep_.tile([128, 512], BF16, tag="ex", bufs=4,
                                              name=f"ex{th}{b}{hh}{kt}")
                                nc.scalar.activation(ex, scps[hh], AF.Exp)
                                # multiply in exp(bias): reversed window read
                                ebr = bass.AP(
                                    tensor=eb.tensor,
                                    offset=eb.offset + ((hh * 4 + kt) * ETW + 512),
                                    ap=[[eb.ap[0][0], 128], [-1, 512]])
                                exb = ep_.tile([128, 512], BF16, tag="exb", bufs=8,
                                               name=f"exb{th}{b}{hh}{kt}")
                                eng = nc.vector if (kt % 2 == 0) else nc.gpsimd
                                eng.tensor_mul(exb, ex, ebr)
                                exbs[(hh, kt)] = exb
                        # ctx: the two heads write disjoint column-halves ->
                        # concurrent via tile_position col groups
                        sps = [psS.tile([1, 512], F32, tag="ps", name=f"s{th}{b}{hh}")
                               for hh in range(2)]
                        for kt in range(4):
                            for hh in range(2):
                                hi = 2 * th + hh
                                ro = hh * 64
                                nc.tensor.matmul(
                                    cps[ro:ro + 64, :],
                                    qkv[:, 16 + b * 4 + kt,
                                        hi * 64:(hi + 1) * 64], exbs[(hh, kt)],
                                    start=(kt == 0), stop=(kt == 3),
                                    tile_position=(0, ro))
                            for hh in range(2):
                                nc.tensor.matmul(sps[hh], ones_bf, exbs[(hh, kt)],
                                                 start=(kt == 0), stop=(kt == 3))
                        rinvs = []
                        for hh in range(2):
                            rinv = ap_.tile([1, 512], BF16, tag="ri",
                                            name=f"ri{th}{b}{hh}")
                            nc.vector.reciprocal(rinv, sps[hh])
                            rinvs.append(rinv)
                        for hh in range(2):
                            ro = hh * 64
                            nc.tensor.matmul(rbp[ro:ro + 64, :], ones1b, rinvs[hh],
                                             start=True, stop=True,
                                             tile_position=(0, ro))
                        rbs = ap_.tile([128, 512], F32, tag="rbs", name=f"rs{th}{b}")
                        nc.scalar.copy(rbs, rbp)
                        nc.vector.tensor_mul(
                            ctxT[:, th, b * 512:(b + 1) * 512], cps, rbs)

            # ---- out-projection + residual -> out1 (slot A)
            out1 = ap_pool.tile([128, DT, NT], F32, tag="A", name=f"out1{i}")
            bo_sb = load_bias_row(ob_d["bo"], i, D, f"bo{i}") if use_obias else None
            for hf in range(2):
                wt = wp.tile([128, DT, 512], BF16, tag="w", name=f"wo{i}{hf}")
                nc.sync.dma_start(out=wt, in_=wqkv_d["wo"][i, hf])
                for m in range(4):
                    mg = hf * 4 + m
                    for ch in range(2):
                        pps = psA.tile([128, 512], F32, tag="pa", name=f"po{mg}{ch}")
                        for k in range(DT):
                            nc.tensor.matmul(
                                pps, wt[:, k, m * 128:(m + 1) * 128],
                                ctxT[:, k, ch * 512:(ch + 1) * 512],
                                start=(k == 0), stop=(k == DT - 1))
                        dsl = out1[:, mg, ch * 512:(ch + 1) * 512]
                        if use_obias:
                            t = psA.tile([128, 512], F32, tag="pa", name=f"ob{mg}{ch}")
                            nc.scalar.activation(t, pps, AF.Identity,
                                                 bias=bo_sb[:, mg:mg + 1])
                            nc.vector.tensor_add(dsl, t,
                                                 h[:, mg, ch * 512:(ch + 1) * 512])
                        else:
                            nc.vector.tensor_add(dsl, pps,
                                                 h[:, mg, ch * 512:(ch + 1) * 512])

            # ---- LN2 -> xn2 (slot B)
            xn2 = bp_pool.tile([128, DT, NT], BF16, tag="B", name=f"xn2{i}")
            layernorm(out1, xn2, i, ln2g_d if use_ln2 else None,
                      ln2b_d if use_ln2 else None, use_ln2, f"b{i}")

            # ---- h += out1  (h becomes h_old + out1 = 2h + attn)
            for d in range(DT):
                eng = nc.vector if (d % 2 == 0) else nc.gpsimd
                eng.tensor_add(h[:, d, :], h[:, d, :], out1[:, d, :])

            # ---- FFN: h += ffn(xn2); weights loaded once, used for both ch
            b1_sb = load_bias_row(pb_d["b1"], i, F, f"b1{i}") if use_pbias else None
            b2_sb = load_bias_row(ob_d["b2"], i, D, f"b2{i}") if use_obias else None
            g = ap_pool.tile([128, FT, NT], BF16, tag="A", name=f"g{i}")
            for fm in range(FT):
                wt = wp.tile([128, DT, 128], BF16, tag="w", name=f"w1{i}{fm}")
                nc.sync.dma_start(out=wt, in_=w1_d[i, fm])
                for ch in range(2):
                    pps = psA.tile([128, 512], F32, tag="pa", name=f"pf{ch}{fm}")
                    for k in range(DT):
                        nc.tensor.matmul(pps, wt[:, k, :],
                                         xn2[:, k, ch * 512:(ch + 1) * 512],
                                         start=(k == 0), stop=(k == DT - 1))
                    gsl = g[:, fm, ch * 512:(ch + 1) * 512]
                    if use_pbias:
                        nc.scalar.activation(gsl, pps, AF.Gelu,
                                             bias=b1_sb[:, fm:fm + 1])
                    else:
                        nc.scalar.activation(gsl, pps, AF.Gelu)
            for dm in range(DT):
                wt = wp.tile([128, FT, 128], BF16, tag="w", name=f"w2{i}{dm}")
                nc.sync.dma_start(out=wt, in_=w2_d[i, dm])
                for ch in range(2):
                    pps = psA.tile([128, 512], F32, tag="pa", name=f"pg{ch}{dm}")
                    for kf in range(FT):
                        nc.tensor.matmul(pps, wt[:, kf, :],
                                         g[:, kf, ch * 512:(ch + 1) * 512],
                                         start=(kf == 0), stop=(kf == FT - 1))
                    hsl = h[:, dm, ch * 512:(ch + 1) * 512]
                    if use_obias:
                        t = psA.tile([128, 512], F32, tag="pa", name=f"o2{ch}{dm}")
                        nc.scalar.activation(t, pps, AF.Identity,
                                             bias=b2_sb[:, dm:dm + 1])
                        nc.vector.tensor_add(hsl, hsl, t)
                    else:
                        nc.vector.tensor_add(hsl, hsl, pps)

        # ---------------- output: transpose h back to token-major -----------
        with ExitStack() as SO:
            op = SO.enter_context(tc.tile_pool(name="outp", bufs=2))
            for b in range(BL):
                for st in range(4):
                    ot = op.tile([128, D], F32, tag="ot", name=f"ot{b}{st}")
                    for half in range(2):
                        ps = psA.tile([128, 512], F32, tag="pa",
                                      name=f"otp{b}{st}{half}")
                        for j in range(4):
                            d = half * 4 + j
                            nc.tensor.transpose(
                                ps[:, j * 128:(j + 1) * 128],
                                h[:, d, b * 512 + st * 128: b * 512 + (st + 1) * 128],
                                im)
                        nc.scalar.copy(ot[:, half * 512:(half + 1) * 512], ps)
                    nc.sync.dma_start(
                        out=out_d[b, st * 128:(st + 1) * 128, :], in_=ot)

    nc.compile()
    return nc


def prepare(inputs):
    """Host-side prep: flags + per-core input maps."""
    x = np.asarray(inputs["x"], dtype=np.float32)
    pe = np.asarray(inputs["pe"], dtype=np.float32).reshape(P, D)[:S]
    bias_table = np.asarray(inputs["bias_table"], dtype=np.float32)

    use_pbias = any(np.any(np.asarray(inputs[k])) for k in ("bq", "bk", "bv", "b1"))
    use_obias = any(np.any(np.asarray(inputs[k])) for k in ("bo", "b2"))
    use_ln1 = (not np.all(np.asarray(inputs["ln1_g"]) == 1.0)) or \
        np.any(np.asarray(inputs["ln1_b"]))
    use_ln2 = (not np.all(np.asarray(inputs["ln2_g"]) == 1.0)) or \
        np.any(np.asarray(inputs["ln2_b"]))
    flags = (use_pbias, use_obias, use_ln1, use_ln2)

    # host-side weight relayout + bf16 cast (pure layout/dtype prep)
    wq = np.asarray(inputs["wq"], dtype=np.float32)
    wk = np.asarray(inputs["wk"], dtype=np.float32)
    wv = np.asarray(inputs["wv"], dtype=np.float32)
    wo = np.asarray(inputs["wo"], dtype=np.float32)
    w1 = np.asarray(inputs["w1"], dtype=np.float32)
    w2 = np.asarray(inputs["w2"], dtype=np.float32)

    def halves(w):  # [L,D,D] -> [L,2,128,DT,512]: [l,hf,p,t,m] = w[l,t*128+p,hf*512+m]
        return np.ascontiguousarray(
            w.reshape(L, DT, 128, 2, 512).transpose(0, 3, 2, 1, 4)).astype(BF)

    # exp(bias) table, transposed to [L,H,TBL], reversed along TBL, with one
    # leading pad element (read but never used): etr[l,h,1+j] = exp(tT[l,h,TBL-1-j])
    tT = bias_table.transpose(0, 2, 1)           # [L,H,TBL]
    etr = np.zeros((L, H, 1 + TBL), dtype=np.float32)
    etr[:, :, 1:] = np.exp(tT[:, :, ::-1])

    base = {
        "pe": np.ascontiguousarray(pe),
        "wq": halves(wq), "wk": halves(wk), "wv": halves(wv), "wo": halves(wo),
        "w1": np.ascontiguousarray(
            w1.reshape(L, DT, 128, FT, 128).transpose(0, 3, 2, 1, 4)).astype(BF),
        "w2": np.ascontiguousarray(
            w2.reshape(L, FT, 128, DT, 128).transpose(0, 3, 2, 1, 4)).astype(BF),
        "etr": etr.astype(BF),
        "imat": np.eye(128, dtype=np.float32),
    }
    if use_pbias:
        base["b_bq"] = np.asarray(inputs["bq"], np.float32) * np.float32(QSCALE)
        base["b_bk"] = np.asarray(inputs["bk"], np.float32)
        base["b_bv"] = np.asarray(inputs["bv"], np.float32)
        base["b_b1"] = np.asarray(inputs["b1"], np.float32)
    if use_obias:
        base["b_bo"] = np.asarray(inputs["bo"], np.float32)
        base["b_b2"] = np.asarray(inputs["b2"], np.float32)
    if use_ln1:
        base["ln1_g"] = np.asarray(inputs["ln1_g"], np.float32)
        base["ln1_b"] = np.asarray(inputs["ln1_b"], np.float32)
    if use_ln2:
        base["ln2_g"] = np.asarray(inputs["ln2_g"], np.float32)
        base["ln2_b"] = np.asarray(inputs["ln2_b"], np.float32)

    in_maps = []
    for c in range(NCORES):
        m = dict(base)
        m["x"] = np.ascontiguousarray(x[c * BL:(c + 1) * BL])
        in_maps.append(m)
    return flags, in_maps


def get_nc(flags):
    if flags not in _CACHE:
        _CACHE[flags] = _build(flags)
    return _CACHE[flags]


def kernel(**inputs):
    flags, in_maps = prepare(inputs)
    nc = get_nc(flags)
    res = run_bass_kernel_spmd(nc, in_maps, core_ids=list(range(NCORES)))
    out = np.concatenate([r["out"] for r in res.results], axis=0)
    return out.astype(np.float32)
